# revision 14
# baseline (speedup 1.0000x reference)
"""Trainium2 Bass kernel: causal multi-head attention block (B=2, T=4096, C=768, H=12).

Sharding: 8 cores = 2 batches x 4 head-groups (3 heads each). Each core runs
QKV projection + causal flash attention + a partial output projection over its
3 heads' channels. Host sums the 4 partials per batch and adds b_proj.

Score blocks are computed as scores^T[key, query] via matmul(lhsT=kT, rhs=qT)
with q and k produced in [head_size, T] layout by the QKV matmul. Pairs of
64-contraction score matmuls (heads 0/1 row-packed; head 2 over two adjacent
q-blocks) write the two banks of one PSUM pair-tile; exp runs directly from
PSUM on the Act engine (no DVE copy). Softmax denominators come from a 65th
all-ones column in the AV stationary operand. Diagonal blocks restrict
score/exp/mask/AV to the causal column range. QKV/proj work is interleaved
into the attention instruction stream as fillers to keep the PE busy.
"""

import sys

for p in ("/opt/trn_rl_repo",):
    if p not in sys.path:
        sys.path.insert(0, p)

from contextlib import ExitStack

import ml_dtypes
import numpy as np

import concourse.bass as bass  # noqa: F401
import concourse.mybir as mybir
import concourse.tile as tile
from concourse import bacc
from concourse.bass_utils import run_bass_kernel_spmd

BF16 = ml_dtypes.bfloat16
F32 = np.float32

B, T, C = 2, 4096, 768
NH, HS = 12, 64
SCALE = HS**-0.5
HPC = 3  # heads per core
N_CORES = 8
P = 128
TQ = 512  # query block width
NQP = T // (2 * TQ)  # q-block pairs = 4
CCH = C // P  # contraction chunks over embed dim = 6
NV = HPC * (HS + 1)  # v columns incl. ones = 195

_DT_BF16 = mybir.dt.bfloat16
_DT_F32 = mybir.dt.float32

_NC = None  # cached compiled Bass module


def _build_bass(repeat=1, loop_reps=0):
    nc = bacc.Bacc("TRN2", target_bir_lowering=False)

    xT_d = nc.dram_tensor("xT", [C, T], _DT_BF16, kind="ExternalInput")
    wqk_d = nc.dram_tensor("wqk", [C, 384], _DT_BF16, kind="ExternalInput")
    wv_d = nc.dram_tensor("wv", [C, NV], _DT_BF16, kind="ExternalInput")
    wproj_d = nc.dram_tensor("wproj", [256, C], _DT_BF16, kind="ExternalInput")
    bqk_d = nc.dram_tensor("bqk", [P, 3], _DT_F32, kind="ExternalInput")
    bv_d = nc.dram_tensor("bv", [P, NV], _DT_F32, kind="ExternalInput")
    mask_d = nc.dram_tensor("mask", [P, P], _DT_BF16, kind="ExternalInput")
    out_d = nc.dram_tensor("out", [T, C], _DT_F32, kind="ExternalOutput")

    with tile.TileContext(nc) as tc, ExitStack() as ctx:
        const = ctx.enter_context(tc.tile_pool(name="const", bufs=1))
        sb = ctx.enter_context(tc.tile_pool(name="work_sb", bufs=4))
        sbd = ctx.enter_context(tc.tile_pool(name="stage_sb", bufs=6))

        env = {
            "xT_sb": const.tile([P, CCH, T], _DT_BF16, tag="xT", name="xT"),
            "wqk_sb": const.tile([P, CCH, 384], _DT_BF16, tag="wqk", name="wqk"),
            "wv_sb": const.tile([P, CCH, NV], _DT_BF16, tag="wv", name="wv"),
            "wproj_sb": const.tile([P, 2, C], _DT_BF16, tag="wproj", name="wproj"),
            "bqk_sb": const.tile([P, 3], _DT_F32, tag="bqk", name="bqk"),
            "bv_sb": const.tile([P, NV], _DT_F32, tag="bv", name="bv"),
            "mask_sb": const.tile([P, P], _DT_BF16, tag="mask", name="mask"),
            # qT/kT chunk 0 rows = [h0 ; h1]; chunk 1 rows 0:64 = h2
            "qT_sb": const.tile([P, 2, T], _DT_BF16, tag="qT", name="qT"),
            "kT_sb": const.tile([P, 2, T], _DT_BF16, tag="kT", name="kT"),
            "v_sb": const.tile([P, T // P, NV], _DT_BF16, tag="v", name="v"),
            "aoT_sb": const.tile([P, 2, T], _DT_BF16, tag="aoT", name="aoT"),
            "sb": sb,
            "sbd": sbd,
            "out_d": out_d,
            "xT_r": xT_d[:].rearrange("(c p) t -> p c t", p=P),
        }

        nc.sync.dma_start(env["wqk_sb"][:], wqk_d[:].rearrange("(c p) m -> p c m", p=P))
        nc.sync.dma_start(env["bqk_sb"][:], bqk_d[:])
        nc.sync.dma_start(env["wv_sb"][:], wv_d[:].rearrange("(c p) m -> p c m", p=P))
        nc.sync.dma_start(env["bv_sb"][:], bv_d[:])
        nc.sync.dma_start(env["mask_sb"][:], mask_d[:])
        nc.sync.dma_start(
            env["wproj_sb"][:], wproj_d[:].rearrange("(k p) n -> p k n", p=P)
        )

        if loop_reps:
            with tc.For_i(0, loop_reps, 1):
                _emit_pipeline(nc, tc, env)
        else:
            for _rep in range(repeat):
                _emit_pipeline(nc, tc, env)

    nc.compile()
    return nc


def _emit_pipeline(nc, tc, env):
    sb = env["sb"]
    sbd = env["sbd"]
    xT_sb = env["xT_sb"]
    wqk_sb = env["wqk_sb"]
    wv_sb = env["wv_sb"]
    wproj_sb = env["wproj_sb"]
    bqk_sb = env["bqk_sb"]
    bv_sb = env["bv_sb"]
    mask_sb = env["mask_sb"]
    qT_sb = env["qT_sb"]
    kT_sb = env["kT_sb"]
    v_sb = env["v_sb"]
    aoT_sb = env["aoT_sb"]
    xT_r = env["xT_r"]
    out_d = env["out_d"]

    EXP = mybir.ActivationFunctionType.Exp
    MULT = mybir.AluOpType.mult
    BYPASS = mybir.AluOpType.bypass

    with (
        tc.tile_pool(name="sc_ps", bufs=2, space="PSUM") as sc_ps,
        tc.tile_pool(name="av_ps", bufs=2, space="PSUM") as av_ps,
        tc.tile_pool(name="mm_ps", bufs=2, space="PSUM") as mm_ps,
    ):
        fillers = []  # deque of thunks interleaved into attend's PE stream

        def emit_xt_dma(tt):
            tsl = slice(tt * TQ, (tt + 1) * TQ)
            for c in range(CCH):
                nc.sync.dma_start(xT_sb[:, c, tsl], xT_r[:, c, tsl])

        def qkv_tile_fillers(tt):
            """Per q/k/v tile: wqk m-groups + v groups, split into half-units
            (3 matmuls each) so filler insertion stays fine-grained."""
            tsl = slice(tt * TQ, (tt + 1) * TQ)
            state = {}

            def qk_half(m, half):
                def emit():
                    if half == 0:
                        state[m] = mm_ps.tile([P, TQ], _DT_F32, tag="small", name="qkps")
                    ps = state[m]
                    for c in range(3 * half, 3 * half + 3):
                        nc.tensor.matmul(
                            ps,
                            wqk_sb[:, c, m * P : (m + 1) * P],
                            xT_sb[:, c, tsl],
                            start=(c == 0),
                            stop=(c == CCH - 1),
                        )
                    if half == 0:
                        return
                    if m == 0:
                        nc.vector.tensor_scalar_add(
                            qT_sb[:, 0, tsl], ps, bqk_sb[:, 0:1]
                        )
                    elif m == 1:
                        nc.vector.tensor_scalar_add(
                            kT_sb[:, 0, tsl], ps, bqk_sb[:, 1:2]
                        )
                    else:
                        # m2 rows: [k2 ; q2]
                        nc.vector.tensor_scalar_add(
                            kT_sb[0:64, 1, tsl], ps[0:64, :], bqk_sb[0:64, 2:3]
                        )
                        nc.vector.tensor_scalar_add(
                            qT_sb[64:P, 1, tsl], ps[64:P, :], bqk_sb[64:P, 2:3]
                        )
                        # shift q2 to rows 0:64 so scores pair with k2 rows 0:64
                        nc.sync.dma_start(qT_sb[0:64, 1, tsl], qT_sb[64:P, 1, tsl])

                return emit

            def v_half(t4, half):
                def emit():
                    tch = tt * (TQ // P) + t4
                    if half == 0:
                        psv = mm_ps.tile([P, TQ], _DT_F32, tag="small", name="vps")
                        state["v", t4] = psv = psv[:, :NV]
                    psv = state["v", t4]
                    for c in range(3 * half, 3 * half + 3):
                        nc.tensor.matmul(
                            psv,
                            xT_sb[:, c, tch * P : (tch + 1) * P],
                            wv_sb[:, c, :],
                            start=(c == 0),
                            stop=(c == CCH - 1),
                        )
                    if half == 1:
                        nc.vector.tensor_add(v_sb[:, tch], psv, bv_sb[:])

                return emit

            units = []
            for m in range(3):
                units += [qk_half(m, 0), qk_half(m, 1)]
            for t4 in range(4):
                units += [v_half(t4, 0), v_half(t4, 1)]
            return units

        def proj_fillers(tt_lo, tt_hi, act_copies=False):
            def proj_unit(tt, nn):
                def emit():
                    t0 = tt * P
                    nsl = slice(nn * 384, (nn + 1) * 384)
                    pp = mm_ps.tile([P, TQ], _DT_F32, tag="small", name="pj")
                    pp = pp[:, :384]
                    nc.tensor.matmul(
                        pp, aoT_sb[:, 0, t0 : t0 + P], wproj_sb[:, 0, nsl],
                        start=True, stop=False,
                    )
                    nc.tensor.matmul(
                        pp, aoT_sb[0:64, 1, t0 : t0 + P], wproj_sb[0:64, 1, nsl],
                        start=False, stop=True,
                    )
                    ot = sb.tile([P, 384], _DT_F32, tag="ot", name="ot")
                    if act_copies and nn == 0:
                        nc.scalar.copy(ot, pp)
                    else:
                        nc.vector.tensor_copy(out=ot, in_=pp)
                    nc.sync.dma_start(out_d[t0 : t0 + P, nsl], ot)

                return emit

            return [proj_unit(tt, nn) for tt in range(tt_lo, tt_hi) for nn in range(2)]

        def pop_filler():
            if fillers:
                fillers.pop(0)()

        def flush_fillers():
            while fillers:
                fillers.pop(0)()

        def normalize(avt, h, tq0):
            """aoT[.., tq0:tq0+TQ] = avt[0:64] / avt[64] (denominator row)."""
            rc = sb.tile([1, TQ], _DT_F32, tag="rc", name="rc")
            nc.vector.reciprocal(rc, avt[64:65, :])
            bcs = sb.tile([64, TQ], _DT_F32, tag="bcs", name="bcs")
            nc.gpsimd.partition_broadcast(bcs[:], rc[:], channels=64)
            chunk, r0 = {0: (0, 0), 1: (0, 64), 2: (1, 0)}[h]
            nc.vector.scalar_tensor_tensor(
                out=aoT_sb[r0 : r0 + 64, chunk, tq0 : tq0 + TQ],
                in0=avt[0:64, :],
                scalar=1.0,
                in1=bcs[:],
                op0=BYPASS,
                op1=MULT,
            )

        def attend(
            hrows, hchunk, rows_lo, rows_hi, tq_lo, tq_hi, n_lo, n_hi,
            early_lo_hook=None,
        ):
            """Two score streams sharing PSUM pair-tiles per key chunk.

            Stream 'lo' uses qT/kT partition rows rows_lo (q-block at tq_lo,
            n_lo key chunks); stream 'hi' uses rows_hi (tq_hi, n_hi chunks).
            hrows: (head of lo stream, head of hi stream) for v columns.
            Diagonal chunks restrict columns to the causal range.
            """
            pop_filler()
            pop_filler()
            av_lo = av_ps.tile([65, TQ], _DT_F32, tag="av", name="av0")
            av_hi = av_ps.tile([65, TQ], _DT_F32, tag="av", name="av1")
            kl, ql = rows_lo
            kh, qh = rows_hi
            pend = None  # (av matmul args for chunk c-1) — one-iter pipeline

            def emit_av(c, col_lo, col_hi, lo_act, pr):
                if lo_act:
                    nc.tensor.matmul(
                        av_lo[:, col_lo:],
                        v_sb[:, c, hrows[0] * 65 : hrows[0] * 65 + 65],
                        pr[:, 0, col_lo:],
                        start=(c == 0),
                        stop=(c == n_lo - 1),
                    )
                nc.tensor.matmul(
                    av_hi[:, col_hi:],
                    v_sb[:, c, hrows[1] * 65 : hrows[1] * 65 + 65],
                    pr[:, 1, col_hi:],
                    start=(c == 0),
                    stop=(c == n_hi - 1),
                )

            for c in range(n_hi):
                lo_act = c < n_lo
                lo_diag = lo_act and c >= n_lo - 4
                hi_diag = c >= n_hi - 4
                col_lo = (c - (n_lo - 4)) * P if lo_diag else 0
                col_hi = (c - (n_hi - 4)) * P if hi_diag else 0
                ksl = slice(c * P, (c + 1) * P)
                scp = sc_ps.tile([P, 2, TQ], _DT_F32, tag="sc", name="sc")
                if lo_act:
                    nc.tensor.matmul(
                        scp[:, 0, col_lo:],
                        kT_sb[kl : kl + 64, hchunk, ksl],
                        qT_sb[ql : ql + 64, hchunk, tq_lo + col_lo : tq_lo + TQ],
                        start=True,
                        stop=True,
                    )
                nc.tensor.matmul(
                    scp[:, 1, col_hi:],
                    kT_sb[kh : kh + 64, hchunk, ksl],
                    qT_sb[qh : qh + 64, hchunk, tq_hi + col_hi : tq_hi + TQ],
                    start=True,
                    stop=True,
                )
                pr = sbd.tile([P, 2, TQ], _DT_BF16, tag="pr", name="pr")
                if lo_act and col_lo == col_hi:
                    nc.scalar.activation(
                        pr[:, :, col_lo:], scp[:, :, col_lo:], EXP, scale=SCALE
                    )
                else:
                    if lo_act:
                        nc.scalar.activation(
                            pr[:, 0, col_lo:], scp[:, 0, col_lo:], EXP, scale=SCALE
                        )
                    nc.scalar.activation(
                        pr[:, 1, col_hi:], scp[:, 1, col_hi:], EXP, scale=SCALE
                    )
                if lo_diag:
                    nc.vector.tensor_mul(
                        pr[:, 0, col_lo : col_lo + P],
                        pr[:, 0, col_lo : col_lo + P],
                        mask_sb[:],
                    )
                if hi_diag:
                    nc.vector.tensor_mul(
                        pr[:, 1, col_hi : col_hi + P],
                        pr[:, 1, col_hi : col_hi + P],
                        mask_sb[:],
                    )
                if pend is not None:
                    emit_av(*pend)
                    if pend[0] == n_lo - 1 and n_lo < n_hi:
                        # lo stream finished early (head-2 attends): normalize
                        # now so dependent proj work can be staged mid-attend
                        normalize(av_lo, 2, tq_lo)
                        av_lo = None
                        if early_lo_hook is not None:
                            early_lo_hook()
                pend = (c, col_lo, col_hi, lo_act, pr)
                if fillers and (len(fillers) >= n_hi - c or c % 2 == 0):
                    pop_filler()
            emit_av(*pend)
            if av_lo is not None:
                normalize(av_lo, hrows[0] if hrows[0] != hrows[1] else 2, tq_lo)
            normalize(av_hi, hrows[1] if hrows[0] != hrows[1] else 2, tq_hi)

        # ---- pipeline: qkv tile 0 inline, then interleaved attend + fillers.
        # The filler deque carries across attend boundaries; ordering is safe
        # because attend(qb) touches kT/v chunks of tile t only at chunk
        # indices >= 4t, well after the staged units for tile t have drained.
        emit_xt_dma(0)
        for f in qkv_tile_fillers(0):
            f()
        for qp in range(NQP):
            qb0, qb1 = 2 * qp, 2 * qp + 1
            nch0, nch1 = 4 * (qb0 + 1), 4 * (qb1 + 1)
            # heads 0/1 on q-block qb0; stage qkv tile qb1 (+ 2nd half of the
            # previous pair's proj)
            emit_xt_dma(qb1)
            fillers.extend(qkv_tile_fillers(qb1))
            if qp > 0:
                fillers.extend(proj_fillers(8 * (qp - 1) + 4, 8 * qp))
            attend((0, 1), 0, (0, 0), (64, 64), qb0 * TQ, qb0 * TQ, nch0, nch0)
            # heads 0/1 on q-block qb1; stage qkv tile qb1+1
            if qp < NQP - 1:
                emit_xt_dma(qb1 + 1)
                fillers.extend(qkv_tile_fillers(qb1 + 1))
            attend((0, 1), 0, (0, 0), (64, 64), qb1 * TQ, qb1 * TQ, nch1, nch1)
            # head 2 over the two adjacent q-blocks (both on rows 0:64); once
            # its lo stream normalizes, this pair's first proj half is ready
            attend(
                (2, 2), 1, (0, 0), (0, 0), qb0 * TQ, qb1 * TQ, nch0, nch1,
                early_lo_hook=(
                    lambda lo=8 * qp: fillers.extend(proj_fillers(lo, lo + 4))
                ),
            )
        flush_fillers()
        for f in proj_fillers(8 * NQP - 4, 8 * NQP, act_copies=True):
            f()


def _get_nc():
    global _NC
    if _NC is None:
        _NC = _build_bass()
    return _NC


def _core_inputs(x, w_attn, b_attn, core):
    """Host-side shard prep for one core."""
    b, g = divmod(core, 4)
    heads = [HPC * g + i for i in range(HPC)]
    h0, h1, h2 = heads

    xT = np.ascontiguousarray(x[b].T).astype(BF16)  # [C, T]

    def rows(base, h):
        return w_attn[base + h * HS : base + (h + 1) * HS]

    def bias(base, h):
        return b_attn[base + h * HS : base + (h + 1) * HS]

    # wqk col blocks: [q_h0 q_h1 | k_h0 k_h1 | k_h2 q_h2]
    blocks = [
        rows(0, h0), rows(0, h1),
        rows(C, h0), rows(C, h1),
        rows(C, h2), rows(0, h2),
    ]
    wqk = np.ascontiguousarray(np.concatenate(blocks, 0).T.astype(BF16))  # [C, 384]
    bias_chunks = [
        np.concatenate([bias(0, h0), bias(0, h1)]),
        np.concatenate([bias(C, h0), bias(C, h1)]),
        np.concatenate([bias(C, h2), bias(0, h2)]),
    ]
    bqk = np.stack(bias_chunks, 1).astype(F32)  # [128, 3]

    # wv cols: per head [v_h (64) | zero]; bias has 1.0 in the ones slot
    vblocks = []
    bv = np.zeros(NV, F32)
    for i, h in enumerate(heads):
        vblocks += [rows(2 * C, h), np.zeros((1, C), w_attn.dtype)]
        bv[i * 65 : i * 65 + HS] = bias(2 * C, h)
        bv[i * 65 + HS] = 1.0
    wv = np.ascontiguousarray(np.concatenate(vblocks, 0).T.astype(BF16))  # [C, 195]
    bv = np.broadcast_to(bv, (P, NV)).astype(F32)

    return xT, wqk, bqk, wv, bv, heads, b


def _mask_arr():
    p = np.arange(P)[:, None]
    n = np.arange(P)[None, :]
    return np.ascontiguousarray((n >= p).astype(BF16))  # [128, 128]


def _prep_in_maps(x, w_attn, b_attn, w_proj):
    mask = _mask_arr()
    in_maps = []
    for core in range(N_CORES):
        xT, wqk, bqk, wv, bv, heads, b = _core_inputs(x, w_attn, b_attn, core)
        h0, h1, h2 = heads
        c0 = np.concatenate(
            [
                w_proj[:, h0 * HS : (h0 + 1) * HS].T,
                w_proj[:, h1 * HS : (h1 + 1) * HS].T,
            ],
            0,
        )  # [128, C]
        c1 = np.concatenate(
            [w_proj[:, h2 * HS : (h2 + 1) * HS].T, np.zeros((64, C), F32)], 0
        )  # [128, C]
        wproj = np.ascontiguousarray(np.concatenate([c0, c1], 0).astype(BF16))
        in_maps.append(
            {
                "xT": xT,
                "wqk": wqk,
                "wv": wv,
                "wproj": wproj,
                "bqk": bqk,
                "bv": bv,
                "mask": mask,
            }
        )
    return in_maps


def _run(inputs, trace=False, **kw):
    x = np.asarray(inputs["x"], F32)
    w_attn = np.asarray(inputs["w_attn"], F32)
    b_attn = np.asarray(inputs["b_attn"], F32)
    w_proj = np.asarray(inputs["w_proj"], F32)
    b_proj = np.asarray(inputs["b_proj"], F32)

    nc = _get_nc()
    in_maps = _prep_in_maps(x, w_attn, b_attn, w_proj)
    res = run_bass_kernel_spmd(
        nc, in_maps, core_ids=list(range(N_CORES)), trace=trace, **kw
    )
    out = np.zeros((B, T, C), F32)
    for core in range(N_CORES):
        out[core // 4] += res.results[core]["out"]
    out += b_proj
    return out, res


def kernel(**inputs):
    out, _ = _run(inputs)
    return out
